# revision 36
# baseline (speedup 1.0000x reference)
"""Multi-head attention + residual + LayerNorm on 8 Trainium2 NeuronCores.

Reference computation (B=2, S=2048, D=1024, H=16, HD=64):
    q = query @ Wq + bq ; k = key @ Wk + bk ; v = value @ Wv + bv   (per-head)
    scores = q k^T / sqrt(HD), masked (-inf where mask), softmax
    att = scores @ v ; out = att @ Wo + bo
    y = LayerNorm(query + out)   (std ddof=1, denom = std + 1e-6)

Sharding:
  Launch 1: 8 cores = 2 batches x 4 head-groups (4 heads/core).
    Each core computes its heads' attention output TRANSPOSED
    (att^T [4*64, S], unnormalized) plus softmax row-sums.
    Layout trick: scores are computed transposed (S^T[sk, sq]) so that
    no on-device transposes are needed anywhere; softmax row-sums come
    free from a ones-column appended to V in the P@V matmul.
    QKV projection of head-pair 1 is interleaved into the attention
    stream of pair 0 so the in-order PE never idles on a serial
    projection prologue.  Mask tiles are prefetched one block ahead
    (uint8 in DRAM, SWDGE-cast to bf16) so block boundaries don't stall
    the PE and re-throttle the HAM clock.
  Launch 2: 8 cores = 2 batches x 4 seq-quarters (512 rows/core).
    Rowsums arrive pre-replicated from the host ([128, KC, 512]); the
    reciprocal + normalize run on-device.  Out-proj accumulates c-outer
    into all 8 PSUM banks so matmuls start while weights stream in.
    LayerNorm uses activation accum_out / tensor_tensor_reduce to get
    row stats for free.
"""

import numpy as np
import ml_dtypes

import concourse.bass as bass
import concourse.tile as tile
from concourse import bacc, mybir
from concourse.bass_utils import run_bass_kernel_spmd

BF16 = ml_dtypes.bfloat16
F8 = ml_dtypes.float8_e4m3fn
WSCALE = 16.0  # fp8 weight/bias prescale (undone via Wo and exp scale)
F32 = np.float32
dt = mybir.dt

B, S, D, H, HD = 2, 2048, 1024, 16, 64
NCORES = 8
HPC = H // 4  # heads per core in launch 1 (4)
EPS = 1e-6
KC = D // 128  # 8 contraction chunks over D
NB = S // 512  # 4 blocks of 512 over sq
SKC = S // 128  # 16 chunks of 128 over sk
SQR = S // 4  # 512 rows per core in launch 2

AF = mybir.ActivationFunctionType
ALU = mybir.AluOpType
AX = mybir.AxisListType

# set by test harness to profile; LAST_EXEC_NS filled per launch when tracing
TRACE = False
LAST_EXEC_NS = []

_CACHE = {}
ATT_ORDER = [(0, 0), (1, 0), (0, 1), (1, 1), (2, 0), (2, 1), (3, 0), (3, 1)]
PAIR1_AT = (0, 0)  # pair-1 in prologue
import os
MASK_U8 = os.environ.get("MASK_U8", "1") == "1"  # uint8 mask, SWDGE-cast on load


def _emit_launch1(tc, qT, kT, vT, mcT, wq, wk, wv, bq, bk, bv, attT, rs):
    nc = tc.nc
    from contextlib import ExitStack

    with ExitStack() as ctx:
        consts = ctx.enter_context(tc.tile_pool(name="consts", bufs=1))
        proj = ctx.enter_context(tc.tile_pool(name="proj", bufs=1))
        maskp = ctx.enter_context(tc.tile_pool(name="mask", bufs=2))
        rawqk = ctx.enter_context(tc.tile_pool(name="rawqk", bufs=1))

        mcts = {}

        def ensure_mask(nb):
            if nb not in mcts:
                mct = maskp.tile(
                    [128, SKC, 512], dt.bfloat16, tag="mct", name=f"mct{nb}"
                )
                nc.gpsimd.dma_start(
                    mct[:],
                    mcT.rearrange("(c p) s -> p c s", p=128)[
                        :, :, nb * 512 : (nb + 1) * 512
                    ],
                )
                mcts[nb] = mct
            return mcts[nb]

        # --- prologue DMAs, ordered so the first projection matmuls can
        # start as soon as wq + the first qT chunks land ---
        ones_bf = consts.tile([1, 256], dt.bfloat16)
        nc.vector.memset(ones_bf[:], 1.0)

        bq_sb = consts.tile([128, 2], dt.float32)
        nc.sync.dma_start(bq_sb[:], bq.rearrange("(j p) -> p j", p=128))
        bk_sb = consts.tile([128, 2], dt.float32)
        nc.sync.dma_start(bk_sb[:], bk.rearrange("(j p) -> p j", p=128))
        wq_sb = consts.tile([128, KC, 256], dt.float8e4)
        nc.sync.dma_start(wq_sb[:], wq.rearrange("(c p) m -> p c m", p=128))
        wk_sb = consts.tile([128, KC, 256], dt.float8e4)
        nc.sync.dma_start(wk_sb[:], wk.rearrange("(c p) m -> p c m", p=128))
        # load q/k by 512-column groups, ordered so the first attention
        # block's dependencies land first: q(nb0), then ALL of k (scores of
        # block (0,0) sweep every key), then the remaining q groups
        qT_sb = rawqk.tile([128, KC, S], dt.float8e4)
        kT_sb = rawqk.tile([128, KC, S], dt.float8e4)
        qr = qT.rearrange("(c p) s -> p c s", p=128)
        kr = kT.rearrange("(c p) s -> p c s", p=128)

        def load_cols(dst, src, g):
            ss = slice(g * 512, (g + 1) * 512)
            nc.sync.dma_start(dst[:, :, ss], src[:, :, ss])

        bv_sb = consts.tile([1, 256], dt.bfloat16)
        nc.sync.dma_start(bv_sb[:], bv.unsqueeze(0))
        wv_sb = consts.tile([128, KC, 256], dt.float8e4)
        nc.sync.dma_start(wv_sb[:], wv.rearrange("(c p) m -> p c m", p=128))

        # everything bandwidth-critical rides ONE HWDGE FIFO in strict
        # priority order (split queues round-robin per packet, which lets
        # the bulky SWDGE mask stream starve the critical q/k prefix)
        vT_sb = rawqk.tile([128, KC, S], dt.float8e4)
        vr = vT.rearrange("(c p) s -> p c s", p=128)
        load_cols(qT_sb, qr, 0)
        for g in range(NB):
            load_cols(kT_sb, kr, g)
        load_cols(vT_sb, vr, 0)
        for g in range(1, NB):
            load_cols(qT_sb, qr, g)
            load_cols(vT_sb, vr, g)
        # masks (uint8->bf16 casts, SWDGE-only) held back behind kT group
        # 0 via a tiny dependency copy so they don't race the prefix
        gdep = consts.tile([1, 4], dt.float8e4)
        nc.gpsimd.dma_start(gdep[0:1, 0:4], kT_sb[0:1, KC - 1, 508:512])
        ensure_mask(ATT_ORDER[0][0])

        # projected q^T / k^T: [128 partitions = 2 heads, pair, S]
        qTp = proj.tile([128, 2, S], dt.bfloat16)
        kTp = proj.tile([128, 2, S], dt.bfloat16)
        # V with a ones column appended per head: [sk-chunk, HPC, HD+1]
        vext = proj.tile([128, SKC, HPC, HD + 1], dt.bfloat16)
        nc.vector.memset(vext[:], 1.0)  # ones col survives; rest overwritten

        # separate PSUM pools so projections can't hog the scores slots
        # (pool slots are FIFO): pj 2x[128,512]=2 banks, sp 2x[128,1024]=4
        # banks, acc 1x[65,1024]=2 banks -> 8 total.
        psum = ctx.enter_context(tc.tile_pool(name="psum", bufs=2, space="PSUM"))
        pjp = ctx.enter_context(tc.tile_pool(name="pjp", bufs=2, space="PSUM"))

        def proj_one(j, nb, w_sb, x_sb, b_sb, outp, name):
            ps = pjp.tile([128, 512], dt.float32, tag="pj", name=name)
            for cc in range(KC // 2):
                nc.tensor.matmul(
                    ps[:],
                    lhsT=w_sb[:, 2 * cc : 2 * cc + 2, j * 128 : (j + 1) * 128],
                    rhs=x_sb[:, 2 * cc : 2 * cc + 2, nb * 512 : (nb + 1) * 512],
                    start=(cc == 0),
                    stop=(cc == KC // 2 - 1),
                    perf_mode=mybir.MatmulPerfMode.DoubleRow,
                )
            nc.vector.tensor_scalar(
                out=outp[:, j, nb * 512 : (nb + 1) * 512],
                in0=ps[:],
                scalar1=b_sb[:, j : j + 1],
                scalar2=None,
                op0=ALU.add,
            )

        def qk_pair(j, first_order=False):
            order = (
                [("q", 0)]
                + [("k", g) for g in range(NB)]
                + [("q", g) for g in range(1, NB)]
                if first_order
                else [(w, g) for g in range(NB) for w in ("q", "k")]
            )
            for w, nb in order:
                if w == "q":
                    proj_one(j, nb, wq_sb, qT_sb, bq_sb, qTp, "psq")
                else:
                    proj_one(j, nb, wk_sb, kT_sb, bk_sb, kTp, "psk")

        # ---- pair-0 projections (emission order matches DMA arrival) ----
        qk_pair(0, first_order=True)

        # attention pools allocated BEFORE the V pool: otherwise they'd
        # reuse vT's SBUF range and inherit a WAR dependency on the LAST
        # V-proj matmul, serializing all of attention behind V-proj
        pp = ctx.enter_context(tc.tile_pool(name="ptile", bufs=3))
        pmp = ctx.enter_context(tc.tile_pool(name="pmtile", bufs=6))
        accps = ctx.enter_context(tc.tile_pool(name="accp", bufs=1, space="PSUM"))
        osb = ctx.enter_context(tc.tile_pool(name="osb", bufs=2))

        # ---- V projection (all heads) ----
        if True:
            for kk in range(SKC):
                ps = pjp.tile([128, 512], dt.float32, tag="pj", name="psv")
                vps = ps[:, 0:256]
                for cc in range(KC // 2):
                    nc.tensor.matmul(
                        vps,
                        lhsT=vT_sb[:, 2 * cc : 2 * cc + 2, kk * 128 : (kk + 1) * 128],
                        rhs=wv_sb[:, 2 * cc : 2 * cc + 2, :],
                        start=(cc == 0),
                        stop=False,
                        perf_mode=mybir.MatmulPerfMode.DoubleRow,
                    )
                nc.tensor.matmul(
                    vps, lhsT=ones_bf[0:1, 0:128], rhs=bv_sb[:], start=False, stop=True
                )
                nc.vector.tensor_copy(
                    vext[:, kk, :, 0:HD],
                    ps[:, 0:256].rearrange("p (h d) -> p h d", h=HPC),
                )

        # ---- attention, with pair-1 projections interleaved ----
        if True:

            def att(nb, t, nxt):
                mct = ensure_mask(nb)
                if nxt is not None:
                    ensure_mask(nxt)
                # both heads' accumulators in one 2-bank tile: h0 cols
                # 0:512 (bank 0), h1 cols 512:1024 (bank 1)
                acc = accps.tile([65, 1024], dt.float32, tag="acc", name=f"a{nb}_{t}")

                for kk in range(SKC):
                    sp = psum.tile([128, 1024], dt.float32, tag="sp", name="sps")
                    for hi in range(2):
                        nc.tensor.matmul(
                            sp[:, hi * 512 : (hi + 1) * 512],
                            lhsT=kTp[
                                hi * 64 : (hi + 1) * 64,
                                t,
                                kk * 128 : (kk + 1) * 128,
                            ],
                            rhs=qTp[
                                hi * 64 : (hi + 1) * 64,
                                t,
                                nb * 512 : (nb + 1) * 512,
                            ],
                            start=True,
                            stop=True,
                            tile_position=(hi * 64, 0),
                        )
                    p = pp.tile([128, 1024], dt.bfloat16, tag="p")
                    nc.scalar.activation(p[:], sp[:], AF.Exp, scale=0.125 / 256.0)
                    pm = pmp.tile([128, 1024], dt.bfloat16, tag="pm")
                    nc.vector.tensor_mul(
                        pm[:].rearrange("p (h s) -> p h s", h=2),
                        p[:].rearrange("p (h s) -> p h s", h=2),
                        mct[:, kk, :].unsqueeze(1).broadcast_to([128, 2, 512]),
                    )
                    for hi in range(2):
                        h = 2 * t + hi
                        nc.tensor.matmul(
                            acc[:, hi * 512 : (hi + 1) * 512],
                            lhsT=vext[:, kk, h, :],
                            rhs=pm[:, hi * 512 : (hi + 1) * 512],
                            start=(kk == 0),
                            stop=(kk == SKC - 1),
                        )
                # merged evacuation: one cast covers both heads' att rows
                # AND the rowsum row (bf16 rowsums: plenty of rel-err slack)
                ao = osb.tile([65, 1024], dt.bfloat16, tag="ao")
                nc.vector.tensor_copy(ao[:], acc[:])
                for hi in range(2):
                    h = 2 * t + hi
                    nc.sync.dma_start(
                        attT[h * 64 : (h + 1) * 64, nb * 512 : (nb + 1) * 512],
                        ao[0:64, hi * 512 : (hi + 1) * 512],
                    )
                    nc.sync.dma_start(
                        rs[h : h + 1, nb * 512 : (nb + 1) * 512],
                        ao[64:65, hi * 512 : (hi + 1) * 512],
                    )

            for i, (nb_, t_) in enumerate(ATT_ORDER):
                nxt = ATT_ORDER[i + 1][0] if i + 1 < len(ATT_ORDER) else None
                # attention outranks the interleaved projections: the PE
                # runs att work the moment it's ready, proj fills the gaps
                with tc.high_priority():
                    att(nb_, t_, nxt)
                if (nb_, t_) == PAIR1_AT:
                    qk_pair(1)


def _emit_launch2(tc, aT, rsr, wo, bo, resid, gamma, beta, out):
    nc = tc.nc
    from contextlib import ExitStack

    MC = SQR // 128  # 4 chunks of 128 rows

    with ExitStack() as ctx:
        consts = ctx.enter_context(tc.tile_pool(name="consts", bufs=1))
        work = ctx.enter_context(tc.tile_pool(name="work", bufs=4))
        stats = ctx.enter_context(tc.tile_pool(name="stats", bufs=16))
        psp = ctx.enter_context(tc.tile_pool(name="psp", bufs=1, space="PSUM"))

        ones1 = consts.tile([1, 128], dt.bfloat16)
        nc.vector.memset(ones1[:], 1.0)
        bo_sb = consts.tile([1, D], dt.bfloat16)
        nc.sync.dma_start(bo_sb[:], bo.unsqueeze(0))

        # reciprocal softmax denominators arrive host-replicated
        # ([128, KC, SQR] bf16); normalize aT per c-chunk as loads land
        rec_sb = consts.tile([128, KC, SQR], dt.bfloat16)
        aT_raw = consts.tile([128, KC, SQR], dt.bfloat16)
        aT_sb = consts.tile([128, KC, SQR], dt.bfloat16)
        wo_sb = consts.tile([128, KC, D], dt.bfloat16)
        ar = aT.rearrange("(c p) s -> p c s", p=128)
        wr = wo.rearrange("(c p) m -> p c m", p=128)
        for h in range(4):
            cs = slice(h * (KC // 4), (h + 1) * (KC // 4))
            nc.sync.dma_start(rec_sb[:, cs, :], rsr[:, cs, :])
            nc.sync.dma_start(aT_raw[:, cs, :], ar[:, cs, :])
            nc.sync.dma_start(wo_sb[:, cs, :], wr[:, cs, :])
            for c in range(h * (KC // 4), (h + 1) * (KC // 4)):
                nc.vector.tensor_mul(aT_sb[:, c, :], aT_raw[:, c, :], rec_sb[:, c, :])

        res_sb = consts.tile([128, MC, D], dt.bfloat16)
        for m in range(MC):
            nc.gpsimd.dma_start(
                res_sb[:, m, :], resid.rearrange("(m p) d -> p m d", p=128)[:, m, :]
            )
        gam = consts.tile([128, D], dt.float32)
        nc.gpsimd.dma_start(gam[:], gamma.unsqueeze(0).broadcast_to([128, D]))
        bet = consts.tile([128, D], dt.float32)
        nc.gpsimd.dma_start(bet[:], beta.unsqueeze(0).broadcast_to([128, D]))

        # out-proj: m-outer / c-inner, so chunk m=0 finishes right after the
        # last weight chunk lands and its LayerNorm overlaps m=1..3 matmuls
        pss = {}
        for m in range(MC):
            for nbk in range(2):
                pss[(m, nbk)] = psp.tile(
                    [128, 512], dt.float32, tag=f"o{m}{nbk}", name=f"o{m}{nbk}"
                )
        for m in range(MC):
            for c in range(KC):
                for nbk in range(2):
                    nc.tensor.matmul(
                        pss[(m, nbk)][:],
                        lhsT=aT_sb[:, c, m * 128 : (m + 1) * 128],
                        rhs=wo_sb[:, c, nbk * 512 : (nbk + 1) * 512],
                        start=(c == 0),
                        stop=False,
                    )
            for nbk in range(2):
                nc.tensor.matmul(
                    pss[(m, nbk)][:],
                    lhsT=ones1[:],
                    rhs=bo_sb[:, nbk * 512 : (nbk + 1) * 512],
                    start=False,
                    stop=True,
                )

        for m in range(MC):
            x = work.tile([128, D], dt.float32, tag="x")
            sm = stats.tile([128, 2], dt.float32, tag="sm")
            for nbk in range(2):
                # x = psum + residual, with the row-sum produced in the
                # same DVE pass (for the LayerNorm mean); accum_out
                # overwrites, so each chunk gets its own slot
                nc.vector.scalar_tensor_tensor(
                    out=x[:, nbk * 512 : (nbk + 1) * 512],
                    in0=pss[(m, nbk)][:],
                    scalar=1.0,
                    in1=res_sb[:, m, nbk * 512 : (nbk + 1) * 512],
                    op0=ALU.mult,
                    op1=ALU.add,
                    accum_out=sm[:, nbk : nbk + 1],
                )
            mn = stats.tile([128, 1], dt.float32, tag="mn")
            nc.vector.tensor_scalar(
                out=mn[:],
                in0=sm[:, 0:1],
                scalar1=sm[:, 1:2],
                scalar2=-1.0 / D,
                op0=ALU.add,
                op1=ALU.mult,
            )
            xm = work.tile([128, D], dt.float32, tag="xm")
            nc.scalar.activation(xm[:], x[:], AF.Identity, bias=mn[:])
            # Square activation accumulates the variance for free
            scr = work.tile([128, D], dt.float32, tag="scr")
            vs = stats.tile([128, 1], dt.float32, tag="vs")
            nc.scalar.activation(scr[:], xm[:], AF.Square, accum_out=vs[:])
            sd = stats.tile([128, 1], dt.float32, tag="sd")
            nc.scalar.activation(sd[:], vs[:], AF.Sqrt, scale=1.0 / (D - 1))
            nc.vector.tensor_scalar_add(sd[:], sd[:], EPS)
            rc = stats.tile([128, 1], dt.float32, tag="rc")
            nc.vector.reciprocal(rc[:], sd[:])
            # y = (xm * rc) * gamma ; out = y + beta
            y = work.tile([128, D], dt.float32, tag="y")
            nc.vector.scalar_tensor_tensor(
                out=y[:],
                in0=xm[:],
                scalar=rc[:],
                in1=gam[:],
                op0=ALU.mult,
                op1=ALU.mult,
            )
            yo = work.tile([128, D], dt.float32, tag="yo")
            # final +beta on the otherwise-idle gpsimd engine
            nc.gpsimd.tensor_tensor(yo[:], y[:], bet[:], ALU.add)
            nc.sync.dma_start(
                out.rearrange("(m p) d -> p m d", p=128)[:, m, :], yo[:]
            )


def _build_launch1():
    nc = bacc.Bacc("TRN2", debug=False, enable_asserts=False)
    qT = nc.dram_tensor("qT", [D, S], dt.float8e4, kind="ExternalInput").ap()
    kT = nc.dram_tensor("kT", [D, S], dt.float8e4, kind="ExternalInput").ap()
    vT = nc.dram_tensor("vT", [D, S], dt.float8e4, kind="ExternalInput").ap()
    mdt = dt.uint8 if MASK_U8 else dt.bfloat16
    mcT = nc.dram_tensor("mcT", [S, S], mdt, kind="ExternalInput").ap()
    wq = nc.dram_tensor("wq", [D, 256], dt.float8e4, kind="ExternalInput").ap()
    wk = nc.dram_tensor("wk", [D, 256], dt.float8e4, kind="ExternalInput").ap()
    wv = nc.dram_tensor("wv", [D, 256], dt.float8e4, kind="ExternalInput").ap()
    bq = nc.dram_tensor("bq", [256], dt.float32, kind="ExternalInput").ap()
    bk = nc.dram_tensor("bk", [256], dt.float32, kind="ExternalInput").ap()
    bv = nc.dram_tensor("bv", [256], dt.bfloat16, kind="ExternalInput").ap()
    attT = nc.dram_tensor("attT", [256, S], dt.bfloat16, kind="ExternalOutput").ap()
    rs = nc.dram_tensor("rs", [HPC, S], dt.bfloat16, kind="ExternalOutput").ap()
    with tile.TileContext(nc) as tc:
        _emit_launch1(tc, qT, kT, vT, mcT, wq, wk, wv, bq, bk, bv, attT, rs)
    nc.compile()
    return nc


def _build_launch2():
    nc = bacc.Bacc("TRN2", debug=False, enable_asserts=False)
    aT = nc.dram_tensor("aT", [D, SQR], dt.bfloat16, kind="ExternalInput").ap()
    rsr = nc.dram_tensor("rsr", [128, KC, SQR], dt.bfloat16, kind="ExternalInput").ap()
    wo = nc.dram_tensor("wo", [D, D], dt.bfloat16, kind="ExternalInput").ap()
    bo = nc.dram_tensor("bo", [D], dt.bfloat16, kind="ExternalInput").ap()
    resid = nc.dram_tensor("resid", [SQR, D], dt.bfloat16, kind="ExternalInput").ap()
    gamma = nc.dram_tensor("gamma", [D], dt.float32, kind="ExternalInput").ap()
    beta = nc.dram_tensor("beta", [D], dt.float32, kind="ExternalInput").ap()
    out = nc.dram_tensor("out", [SQR, D], dt.float32, kind="ExternalOutput").ap()
    with tile.TileContext(nc) as tc:
        _emit_launch2(tc, aT, rsr, wo, bo, resid, gamma, beta, out)
    nc.compile()
    return nc


def _get(name):
    if name not in _CACHE:
        _CACHE[name] = _build_launch1() if name == "l1" else _build_launch2()
    return _CACHE[name]


def kernel(query, key, value, mask, Wq, bq, Wk, bk, Wv, bv, Wo, bo, gamma, beta):
    global LAST_EXEC_NS
    LAST_EXEC_NS = []
    query = np.asarray(query, dtype=F32)
    key = np.asarray(key, dtype=F32)
    value = np.asarray(value, dtype=F32)
    mask = np.asarray(mask)
    Wq, Wk, Wv, Wo = (np.asarray(a, dtype=F32) for a in (Wq, Wk, Wv, Wo))
    bq, bk, bv, bo = (np.asarray(a, dtype=F32) for a in (bq, bk, bv, bo))
    gamma = np.asarray(gamma, dtype=F32)
    beta = np.asarray(beta, dtype=F32)

    # ---- launch 1: attention, sharded (batch x 4-head-group) ----
    qT = [np.ascontiguousarray(query[b].T.astype(F8)) for b in range(B)]
    kTt = [np.ascontiguousarray(key[b].T.astype(F8)) for b in range(B)]
    vTt = [np.ascontiguousarray(value[b].T.astype(F8)) for b in range(B)]
    if MASK_U8:
        mcT = [np.ascontiguousarray((~mask[b]).T.astype(np.uint8)) for b in range(B)]
    else:
        mcT = [np.ascontiguousarray((~mask[b]).T.astype(BF16)) for b in range(B)]

    in_maps1 = []
    for c in range(NCORES):
        b, g = c // 4, c % 4
        sl = slice(g * 256, (g + 1) * 256)
        in_maps1.append(
            {
                "qT": qT[b],
                "kT": kTt[b],
                "vT": vTt[b],
                "mcT": mcT[b],
                "wq": np.ascontiguousarray((Wq[:, sl] * WSCALE).astype(F8)),
                "wk": np.ascontiguousarray((Wk[:, sl] * WSCALE).astype(F8)),
                "wv": np.ascontiguousarray((Wv[:, sl] * WSCALE).astype(F8)),
                "bq": np.ascontiguousarray(bq[sl] * WSCALE),
                "bk": np.ascontiguousarray(bk[sl] * WSCALE),
                "bv": np.ascontiguousarray((bv[sl] * WSCALE).astype(BF16)),
            }
        )
    nc1 = _get("l1")
    r1 = run_bass_kernel_spmd(nc1, in_maps1, core_ids=list(range(NCORES)), trace=TRACE)
    if TRACE:
        LAST_EXEC_NS.append(r1.exec_time_ns)

    # assemble att^T and rowsums per batch
    attT_full = [
        np.concatenate([r1.results[b * 4 + g]["attT"] for g in range(4)], axis=0)
        for b in range(B)
    ]
    rs_full = [
        np.concatenate([r1.results[b * 4 + g]["rs"] for g in range(4)], axis=0)
        for b in range(B)
    ]

    # ---- launch 2: out-proj + residual + LayerNorm, sharded (batch x seq/4) ----
    wo_bf = np.ascontiguousarray((Wo / WSCALE).astype(BF16))
    bo_bf = np.ascontiguousarray(bo.astype(BF16))
    in_maps2 = []
    for c in range(NCORES):
        b, q = c // 4, c % 4
        sl = slice(q * SQR, (q + 1) * SQR)
        # reciprocal of each head's rowsum, replicated across its 64 aT
        # rows: rsr[p, c, :] = 1 / rs[2c + (p >= 64), :]
        rc_q = (1.0 / rs_full[b][:, sl].astype(F32)).astype(BF16)  # [16, 512]
        rsr = np.ascontiguousarray(
            np.repeat(rc_q, HD, axis=0).reshape(KC, 128, SQR).transpose(1, 0, 2)
        )
        in_maps2.append(
            {
                "aT": np.ascontiguousarray(attT_full[b][:, sl]),
                "rsr": rsr,
                "wo": wo_bf,
                "bo": bo_bf,
                "resid": np.ascontiguousarray(query[b, sl, :].astype(BF16)),
                "gamma": gamma,
                "beta": beta,
            }
        )
    nc2 = _get("l2")
    r2 = run_bass_kernel_spmd(nc2, in_maps2, core_ids=list(range(NCORES)), trace=TRACE)
    if TRACE:
        LAST_EXEC_NS.append(r2.exec_time_ns)

    out = np.empty((B, S, D), dtype=F32)
    for c in range(NCORES):
        b, q = c // 4, c % 4
        out[b, q * SQR : (q + 1) * SQR, :] = r2.results[c]["out"]
    return out


# revision 37
# speedup vs baseline: 1.0347x; 1.0347x over previous
"""Multi-head attention + residual + LayerNorm on 8 Trainium2 NeuronCores.

Reference computation (B=2, S=2048, D=1024, H=16, HD=64):
    q = query @ Wq + bq ; k = key @ Wk + bk ; v = value @ Wv + bv   (per-head)
    scores = q k^T / sqrt(HD), masked (-inf where mask), softmax
    att = scores @ v ; out = att @ Wo + bo
    y = LayerNorm(query + out)   (std ddof=1, denom = std + 1e-6)

Sharding:
  Launch 1: 8 cores = 2 batches x 4 head-groups (4 heads/core).
    Each core computes its heads' attention output TRANSPOSED
    (att^T [4*64, S], unnormalized) plus softmax row-sums.
    Layout trick: scores are computed transposed (S^T[sk, sq]) so that
    no on-device transposes are needed anywhere; softmax row-sums come
    free from a ones-column appended to V in the P@V matmul.
    QKV projection of head-pair 1 is interleaved into the attention
    stream of pair 0 so the in-order PE never idles on a serial
    projection prologue.  Mask tiles are prefetched one block ahead
    (uint8 in DRAM, SWDGE-cast to bf16) so block boundaries don't stall
    the PE and re-throttle the HAM clock.
  Launch 2: 8 cores = 2 batches x 4 seq-quarters (512 rows/core).
    Rowsums arrive pre-replicated from the host ([128, KC, 512]); the
    reciprocal + normalize run on-device.  Out-proj accumulates c-outer
    into all 8 PSUM banks so matmuls start while weights stream in.
    LayerNorm uses activation accum_out / tensor_tensor_reduce to get
    row stats for free.
"""

import numpy as np
import ml_dtypes

import concourse.bass as bass
import concourse.tile as tile
from concourse import bacc, mybir
from concourse.bass_utils import run_bass_kernel_spmd

BF16 = ml_dtypes.bfloat16
F8 = ml_dtypes.float8_e4m3fn
WSCALE = 16.0  # fp8 weight/bias prescale (undone via Wo and exp scale)
F32 = np.float32
dt = mybir.dt

B, S, D, H, HD = 2, 2048, 1024, 16, 64
NCORES = 8
HPC = H // 4  # heads per core in launch 1 (4)
EPS = 1e-6
KC = D // 128  # 8 contraction chunks over D
NB = S // 512  # 4 blocks of 512 over sq
SKC = S // 128  # 16 chunks of 128 over sk
SQR = S // 4  # 512 rows per core in launch 2

AF = mybir.ActivationFunctionType
ALU = mybir.AluOpType
AX = mybir.AxisListType

# set by test harness to profile; LAST_EXEC_NS filled per launch when tracing
TRACE = False
LAST_EXEC_NS = []

_CACHE = {}
ATT_ORDER = [(0, 0), (1, 0), (0, 1), (1, 1), (2, 0), (2, 1), (3, 0), (3, 1)]
PAIR1_AT = (0, 0)  # pair-1 in prologue
import os
MASK_U8 = os.environ.get("MASK_U8", "1") == "1"  # uint8 mask, SWDGE-cast on load


def _emit_launch1(tc, qT, kT, vT, mcT, wq, wk, wv, bq, bk, bv, attT, rs):
    nc = tc.nc
    from contextlib import ExitStack

    with ExitStack() as ctx:
        consts = ctx.enter_context(tc.tile_pool(name="consts", bufs=1))
        proj = ctx.enter_context(tc.tile_pool(name="proj", bufs=1))
        maskp = ctx.enter_context(tc.tile_pool(name="mask", bufs=2))
        rawqk = ctx.enter_context(tc.tile_pool(name="rawqk", bufs=1))

        mcts = {}

        def ensure_mask(nb):
            if nb not in mcts:
                mct = maskp.tile(
                    [128, SKC, 512], dt.bfloat16, tag="mct", name=f"mct{nb}"
                )
                nc.gpsimd.dma_start(
                    mct[:],
                    mcT.rearrange("(c p) s -> p c s", p=128)[
                        :, :, nb * 512 : (nb + 1) * 512
                    ],
                )
                mcts[nb] = mct
            return mcts[nb]

        # --- prologue DMAs, ordered so the first projection matmuls can
        # start as soon as wq + the first qT chunks land ---
        ones_bf = consts.tile([1, 256], dt.bfloat16)
        nc.vector.memset(ones_bf[:], 1.0)

        bq_sb = consts.tile([128, 2], dt.float32)
        nc.sync.dma_start(bq_sb[:], bq.rearrange("(j p) -> p j", p=128))
        bk_sb = consts.tile([128, 2], dt.float32)
        nc.sync.dma_start(bk_sb[:], bk.rearrange("(j p) -> p j", p=128))
        wq_sb = consts.tile([128, KC, 256], dt.float8e4)
        nc.sync.dma_start(wq_sb[:], wq.rearrange("(c p) m -> p c m", p=128))
        wk_sb = consts.tile([128, KC, 256], dt.float8e4)
        nc.sync.dma_start(wk_sb[:], wk.rearrange("(c p) m -> p c m", p=128))
        # load q/k by 512-column groups, ordered so the first attention
        # block's dependencies land first: q(nb0), then ALL of k (scores of
        # block (0,0) sweep every key), then the remaining q groups
        qT_sb = rawqk.tile([128, KC, S], dt.float8e4)
        kT_sb = rawqk.tile([128, KC, S], dt.float8e4)
        qr = qT.rearrange("(c p) s -> p c s", p=128)
        kr = kT.rearrange("(c p) s -> p c s", p=128)

        def load_cols(dst, src, g):
            ss = slice(g * 512, (g + 1) * 512)
            nc.sync.dma_start(dst[:, :, ss], src[:, :, ss])

        bv_sb = consts.tile([1, 256], dt.bfloat16)
        nc.sync.dma_start(bv_sb[:], bv.unsqueeze(0))
        wv_sb = consts.tile([128, KC, 256], dt.float8e4)
        nc.sync.dma_start(wv_sb[:], wv.rearrange("(c p) m -> p c m", p=128))

        # everything bandwidth-critical rides ONE HWDGE FIFO in strict
        # priority order (split queues round-robin per packet, which lets
        # the bulky SWDGE mask stream starve the critical q/k prefix)
        vT_sb = rawqk.tile([128, KC, S], dt.float8e4)
        vr = vT.rearrange("(c p) s -> p c s", p=128)
        load_cols(qT_sb, qr, 0)
        for g in range(NB):
            load_cols(kT_sb, kr, g)
        load_cols(vT_sb, vr, 0)
        for g in range(1, NB):
            load_cols(qT_sb, qr, g)
            load_cols(vT_sb, vr, g)
        # masks (uint8->bf16 casts, SWDGE-only) held back behind kT group
        # 0 via a tiny dependency copy so they don't race the prefix
        gdep = consts.tile([1, 4], dt.float8e4)
        nc.gpsimd.dma_start(gdep[0:1, 0:4], kT_sb[0:1, KC - 1, 508:512])
        ensure_mask(ATT_ORDER[0][0])

        # projected q^T / k^T: [128 partitions = 2 heads, pair, S]
        qTp = proj.tile([128, 2, S], dt.bfloat16)
        kTp = proj.tile([128, 2, S], dt.bfloat16)
        # V with a ones column appended per head: [sk-chunk, HPC, HD+1]
        vext = proj.tile([128, SKC, HPC, HD + 1], dt.bfloat16)
        nc.vector.memset(vext[:], 1.0)  # ones col survives; rest overwritten

        # separate PSUM pools so projections can't hog the scores slots
        # (pool slots are FIFO): pj 2x[128,512]=2 banks, sp 2x[128,1024]=4
        # banks, acc 1x[65,1024]=2 banks -> 8 total.
        psum = ctx.enter_context(tc.tile_pool(name="psum", bufs=2, space="PSUM"))
        pjp = ctx.enter_context(tc.tile_pool(name="pjp", bufs=2, space="PSUM"))

        def proj_one(j, nb, w_sb, x_sb, b_sb, outp, name):
            ps = pjp.tile([128, 512], dt.float32, tag="pj", name=name)
            for cc in range(KC // 2):
                nc.tensor.matmul(
                    ps[:],
                    lhsT=w_sb[:, 2 * cc : 2 * cc + 2, j * 128 : (j + 1) * 128],
                    rhs=x_sb[:, 2 * cc : 2 * cc + 2, nb * 512 : (nb + 1) * 512],
                    start=(cc == 0),
                    stop=(cc == KC // 2 - 1),
                    perf_mode=mybir.MatmulPerfMode.DoubleRow,
                )
            nc.vector.tensor_scalar(
                out=outp[:, j, nb * 512 : (nb + 1) * 512],
                in0=ps[:],
                scalar1=b_sb[:, j : j + 1],
                scalar2=None,
                op0=ALU.add,
            )

        def qk_pair(j, first_order=False):
            order = (
                [("q", 0)]
                + [("k", g) for g in range(NB)]
                + [("q", g) for g in range(1, NB)]
                if first_order
                else [(w, g) for g in range(NB) for w in ("q", "k")]
            )
            for w, nb in order:
                if w == "q":
                    proj_one(j, nb, wq_sb, qT_sb, bq_sb, qTp, "psq")
                else:
                    proj_one(j, nb, wk_sb, kT_sb, bk_sb, kTp, "psk")

        # ---- pair-0 projections (emission order matches DMA arrival) ----
        qk_pair(0, first_order=True)

        # attention pools allocated BEFORE the V pool: otherwise they'd
        # reuse vT's SBUF range and inherit a WAR dependency on the LAST
        # V-proj matmul, serializing all of attention behind V-proj
        pp = ctx.enter_context(tc.tile_pool(name="ptile", bufs=3))
        pmp = ctx.enter_context(tc.tile_pool(name="pmtile", bufs=6))
        accps = ctx.enter_context(tc.tile_pool(name="accp", bufs=1, space="PSUM"))
        osb = ctx.enter_context(tc.tile_pool(name="osb", bufs=2))

        # ---- V projection (all heads) ----
        if True:
            for kk in range(SKC):
                ps = pjp.tile([128, 512], dt.float32, tag="pj", name="psv")
                vps = ps[:, 0:256]
                for cc in range(KC // 2):
                    nc.tensor.matmul(
                        vps,
                        lhsT=vT_sb[:, 2 * cc : 2 * cc + 2, kk * 128 : (kk + 1) * 128],
                        rhs=wv_sb[:, 2 * cc : 2 * cc + 2, :],
                        start=(cc == 0),
                        stop=False,
                        perf_mode=mybir.MatmulPerfMode.DoubleRow,
                    )
                nc.tensor.matmul(
                    vps, lhsT=ones_bf[0:1, 0:128], rhs=bv_sb[:], start=False, stop=True
                )
                nc.vector.tensor_copy(
                    vext[:, kk, :, 0:HD],
                    ps[:, 0:256].rearrange("p (h d) -> p h d", h=HPC),
                )

        # ---- attention, with pair-1 projections interleaved ----
        if True:

            def att(nb, t, nxt):
                mct = ensure_mask(nb)
                if nxt is not None:
                    ensure_mask(nxt)
                # both heads' accumulators in one 2-bank tile: h0 cols
                # 0:512 (bank 0), h1 cols 512:1024 (bank 1)
                acc = accps.tile([65, 1024], dt.float32, tag="acc", name=f"a{nb}_{t}")

                for kk in range(SKC):
                    sp = psum.tile([128, 1024], dt.float32, tag="sp", name="sps")
                    for hi in range(2):
                        nc.tensor.matmul(
                            sp[:, hi * 512 : (hi + 1) * 512],
                            lhsT=kTp[
                                hi * 64 : (hi + 1) * 64,
                                t,
                                kk * 128 : (kk + 1) * 128,
                            ],
                            rhs=qTp[
                                hi * 64 : (hi + 1) * 64,
                                t,
                                nb * 512 : (nb + 1) * 512,
                            ],
                            start=True,
                            stop=True,
                            tile_position=(hi * 64, 0),
                        )
                    p = pp.tile([128, 1024], dt.bfloat16, tag="p")
                    nc.scalar.activation(p[:], sp[:], AF.Exp, scale=0.125 / 256.0)
                    pm = pmp.tile([128, 1024], dt.bfloat16, tag="pm")
                    nc.vector.tensor_mul(
                        pm[:].rearrange("p (h s) -> p h s", h=2),
                        p[:].rearrange("p (h s) -> p h s", h=2),
                        mct[:, kk, :].unsqueeze(1).broadcast_to([128, 2, 512]),
                    )
                    for hi in range(2):
                        h = 2 * t + hi
                        nc.tensor.matmul(
                            acc[:, hi * 512 : (hi + 1) * 512],
                            lhsT=vext[:, kk, h, :],
                            rhs=pm[:, hi * 512 : (hi + 1) * 512],
                            start=(kk == 0),
                            stop=(kk == SKC - 1),
                        )
                # merged evacuation: one cast covers both heads' att rows
                # AND the rowsum row (bf16 rowsums: plenty of rel-err slack)
                ao = osb.tile([65, 1024], dt.bfloat16, tag="ao")
                nc.vector.tensor_copy(ao[:], acc[:])
                for hi in range(2):
                    h = 2 * t + hi
                    nc.sync.dma_start(
                        attT[h * 64 : (h + 1) * 64, nb * 512 : (nb + 1) * 512],
                        ao[0:64, hi * 512 : (hi + 1) * 512],
                    )
                    nc.sync.dma_start(
                        rs[h : h + 1, nb * 512 : (nb + 1) * 512],
                        ao[64:65, hi * 512 : (hi + 1) * 512],
                    )

            for i, (nb_, t_) in enumerate(ATT_ORDER):
                nxt = ATT_ORDER[i + 1][0] if i + 1 < len(ATT_ORDER) else None
                # attention outranks the interleaved projections: the PE
                # runs att work the moment it's ready, proj fills the gaps
                with tc.high_priority():
                    att(nb_, t_, nxt)
                if (nb_, t_) == PAIR1_AT:
                    qk_pair(1)


def _emit_launch2(tc, aT, rsr, wo, bo, resid, gamma, beta, out):
    nc = tc.nc
    from contextlib import ExitStack

    MC = SQR // 128  # 4 chunks of 128 rows

    with ExitStack() as ctx:
        consts = ctx.enter_context(tc.tile_pool(name="consts", bufs=1))
        work = ctx.enter_context(tc.tile_pool(name="work", bufs=4))
        stats = ctx.enter_context(tc.tile_pool(name="stats", bufs=16))
        psp = ctx.enter_context(tc.tile_pool(name="psp", bufs=1, space="PSUM"))

        ones1 = consts.tile([1, 128], dt.bfloat16)
        nc.vector.memset(ones1[:], 1.0)
        bo_sb = consts.tile([1, D], dt.bfloat16)
        nc.sync.dma_start(bo_sb[:], bo.unsqueeze(0))

        # reciprocal softmax denominators arrive host-replicated
        # ([128, KC, SQR] bf16); normalize aT per c-chunk as loads land
        rec_sb = consts.tile([128, KC, SQR], dt.bfloat16)
        aT_raw = consts.tile([128, KC, SQR], dt.bfloat16)
        aT_sb = consts.tile([128, KC, SQR], dt.bfloat16)
        wo_sb = consts.tile([128, KC, D], dt.bfloat16)
        ar = aT.rearrange("(c p) s -> p c s", p=128)
        wr = wo.rearrange("(c p) m -> p c m", p=128)
        for h in range(2):
            cs = slice(h * (KC // 2), (h + 1) * (KC // 2))
            nc.sync.dma_start(rec_sb[:, cs, :], rsr[:, cs, :])
            nc.sync.dma_start(aT_raw[:, cs, :], ar[:, cs, :])
            nc.sync.dma_start(wo_sb[:, cs, :], wr[:, cs, :])
            for c in range(h * (KC // 2), (h + 1) * (KC // 2)):
                nc.vector.tensor_mul(aT_sb[:, c, :], aT_raw[:, c, :], rec_sb[:, c, :])

        res_sb = consts.tile([128, MC, D], dt.bfloat16)
        for m in range(MC):
            nc.gpsimd.dma_start(
                res_sb[:, m, :], resid.rearrange("(m p) d -> p m d", p=128)[:, m, :]
            )
        gam = consts.tile([128, D], dt.float32)
        nc.gpsimd.dma_start(gam[:], gamma.unsqueeze(0).broadcast_to([128, D]))
        bet = consts.tile([128, D], dt.float32)
        nc.gpsimd.dma_start(bet[:], beta.unsqueeze(0).broadcast_to([128, D]))

        # out-proj: m-outer / c-inner, so chunk m=0 finishes right after the
        # last weight chunk lands and its LayerNorm overlaps m=1..3 matmuls
        pss = {}
        for m in range(MC):
            for nbk in range(2):
                pss[(m, nbk)] = psp.tile(
                    [128, 512], dt.float32, tag=f"o{m}{nbk}", name=f"o{m}{nbk}"
                )
        for m in range(MC):
            for c in range(KC):
                for nbk in range(2):
                    nc.tensor.matmul(
                        pss[(m, nbk)][:],
                        lhsT=aT_sb[:, c, m * 128 : (m + 1) * 128],
                        rhs=wo_sb[:, c, nbk * 512 : (nbk + 1) * 512],
                        start=(c == 0),
                        stop=False,
                    )
            for nbk in range(2):
                nc.tensor.matmul(
                    pss[(m, nbk)][:],
                    lhsT=ones1[:],
                    rhs=bo_sb[:, nbk * 512 : (nbk + 1) * 512],
                    start=False,
                    stop=True,
                )

        for m in range(MC):
            x = work.tile([128, D], dt.float32, tag="x")
            sm = stats.tile([128, 2], dt.float32, tag="sm")
            for nbk in range(2):
                # x = psum + residual, with the row-sum produced in the
                # same DVE pass (for the LayerNorm mean); accum_out
                # overwrites, so each chunk gets its own slot
                nc.vector.scalar_tensor_tensor(
                    out=x[:, nbk * 512 : (nbk + 1) * 512],
                    in0=pss[(m, nbk)][:],
                    scalar=1.0,
                    in1=res_sb[:, m, nbk * 512 : (nbk + 1) * 512],
                    op0=ALU.mult,
                    op1=ALU.add,
                    accum_out=sm[:, nbk : nbk + 1],
                )
            mn = stats.tile([128, 1], dt.float32, tag="mn")
            nc.vector.tensor_scalar(
                out=mn[:],
                in0=sm[:, 0:1],
                scalar1=sm[:, 1:2],
                scalar2=-1.0 / D,
                op0=ALU.add,
                op1=ALU.mult,
            )
            xm = work.tile([128, D], dt.float32, tag="xm")
            nc.scalar.activation(xm[:], x[:], AF.Identity, bias=mn[:])
            # Square activation accumulates the variance for free
            scr = work.tile([128, D], dt.float32, tag="scr")
            vs = stats.tile([128, 1], dt.float32, tag="vs")
            nc.scalar.activation(scr[:], xm[:], AF.Square, accum_out=vs[:])
            sd = stats.tile([128, 1], dt.float32, tag="sd")
            nc.scalar.activation(sd[:], vs[:], AF.Sqrt, scale=1.0 / (D - 1))
            nc.vector.tensor_scalar_add(sd[:], sd[:], EPS)
            rc = stats.tile([128, 1], dt.float32, tag="rc")
            nc.vector.reciprocal(rc[:], sd[:])
            # y = (xm * rc) * gamma ; out = y + beta
            y = work.tile([128, D], dt.float32, tag="y")
            nc.vector.scalar_tensor_tensor(
                out=y[:],
                in0=xm[:],
                scalar=rc[:],
                in1=gam[:],
                op0=ALU.mult,
                op1=ALU.mult,
            )
            yo = work.tile([128, D], dt.float32, tag="yo")
            # final +beta on the otherwise-idle gpsimd engine
            nc.gpsimd.tensor_tensor(yo[:], y[:], bet[:], ALU.add)
            nc.sync.dma_start(
                out.rearrange("(m p) d -> p m d", p=128)[:, m, :], yo[:]
            )


def _build_launch1():
    nc = bacc.Bacc("TRN2", debug=False, enable_asserts=False)
    qT = nc.dram_tensor("qT", [D, S], dt.float8e4, kind="ExternalInput").ap()
    kT = nc.dram_tensor("kT", [D, S], dt.float8e4, kind="ExternalInput").ap()
    vT = nc.dram_tensor("vT", [D, S], dt.float8e4, kind="ExternalInput").ap()
    mdt = dt.uint8 if MASK_U8 else dt.bfloat16
    mcT = nc.dram_tensor("mcT", [S, S], mdt, kind="ExternalInput").ap()
    wq = nc.dram_tensor("wq", [D, 256], dt.float8e4, kind="ExternalInput").ap()
    wk = nc.dram_tensor("wk", [D, 256], dt.float8e4, kind="ExternalInput").ap()
    wv = nc.dram_tensor("wv", [D, 256], dt.float8e4, kind="ExternalInput").ap()
    bq = nc.dram_tensor("bq", [256], dt.float32, kind="ExternalInput").ap()
    bk = nc.dram_tensor("bk", [256], dt.float32, kind="ExternalInput").ap()
    bv = nc.dram_tensor("bv", [256], dt.bfloat16, kind="ExternalInput").ap()
    attT = nc.dram_tensor("attT", [256, S], dt.bfloat16, kind="ExternalOutput").ap()
    rs = nc.dram_tensor("rs", [HPC, S], dt.bfloat16, kind="ExternalOutput").ap()
    with tile.TileContext(nc) as tc:
        _emit_launch1(tc, qT, kT, vT, mcT, wq, wk, wv, bq, bk, bv, attT, rs)
    nc.compile()
    return nc


def _build_launch2():
    nc = bacc.Bacc("TRN2", debug=False, enable_asserts=False)
    aT = nc.dram_tensor("aT", [D, SQR], dt.bfloat16, kind="ExternalInput").ap()
    rsr = nc.dram_tensor("rsr", [128, KC, SQR], dt.bfloat16, kind="ExternalInput").ap()
    wo = nc.dram_tensor("wo", [D, D], dt.bfloat16, kind="ExternalInput").ap()
    bo = nc.dram_tensor("bo", [D], dt.bfloat16, kind="ExternalInput").ap()
    resid = nc.dram_tensor("resid", [SQR, D], dt.bfloat16, kind="ExternalInput").ap()
    gamma = nc.dram_tensor("gamma", [D], dt.float32, kind="ExternalInput").ap()
    beta = nc.dram_tensor("beta", [D], dt.float32, kind="ExternalInput").ap()
    out = nc.dram_tensor("out", [SQR, D], dt.float32, kind="ExternalOutput").ap()
    with tile.TileContext(nc) as tc:
        _emit_launch2(tc, aT, rsr, wo, bo, resid, gamma, beta, out)
    nc.compile()
    return nc


def _get(name):
    if name not in _CACHE:
        _CACHE[name] = _build_launch1() if name == "l1" else _build_launch2()
    return _CACHE[name]


def kernel(query, key, value, mask, Wq, bq, Wk, bk, Wv, bv, Wo, bo, gamma, beta):
    global LAST_EXEC_NS
    LAST_EXEC_NS = []
    query = np.asarray(query, dtype=F32)
    key = np.asarray(key, dtype=F32)
    value = np.asarray(value, dtype=F32)
    mask = np.asarray(mask)
    Wq, Wk, Wv, Wo = (np.asarray(a, dtype=F32) for a in (Wq, Wk, Wv, Wo))
    bq, bk, bv, bo = (np.asarray(a, dtype=F32) for a in (bq, bk, bv, bo))
    gamma = np.asarray(gamma, dtype=F32)
    beta = np.asarray(beta, dtype=F32)

    # ---- launch 1: attention, sharded (batch x 4-head-group) ----
    qT = [np.ascontiguousarray(query[b].T.astype(F8)) for b in range(B)]
    kTt = [np.ascontiguousarray(key[b].T.astype(F8)) for b in range(B)]
    vTt = [np.ascontiguousarray(value[b].T.astype(F8)) for b in range(B)]
    if MASK_U8:
        mcT = [np.ascontiguousarray((~mask[b]).T.astype(np.uint8)) for b in range(B)]
    else:
        mcT = [np.ascontiguousarray((~mask[b]).T.astype(BF16)) for b in range(B)]

    in_maps1 = []
    for c in range(NCORES):
        b, g = c // 4, c % 4
        sl = slice(g * 256, (g + 1) * 256)
        in_maps1.append(
            {
                "qT": qT[b],
                "kT": kTt[b],
                "vT": vTt[b],
                "mcT": mcT[b],
                "wq": np.ascontiguousarray((Wq[:, sl] * WSCALE).astype(F8)),
                "wk": np.ascontiguousarray((Wk[:, sl] * WSCALE).astype(F8)),
                "wv": np.ascontiguousarray((Wv[:, sl] * WSCALE).astype(F8)),
                "bq": np.ascontiguousarray(bq[sl] * WSCALE),
                "bk": np.ascontiguousarray(bk[sl] * WSCALE),
                "bv": np.ascontiguousarray((bv[sl] * WSCALE).astype(BF16)),
            }
        )
    nc1 = _get("l1")
    r1 = run_bass_kernel_spmd(nc1, in_maps1, core_ids=list(range(NCORES)), trace=TRACE)
    if TRACE:
        LAST_EXEC_NS.append(r1.exec_time_ns)

    # assemble att^T and rowsums per batch
    attT_full = [
        np.concatenate([r1.results[b * 4 + g]["attT"] for g in range(4)], axis=0)
        for b in range(B)
    ]
    rs_full = [
        np.concatenate([r1.results[b * 4 + g]["rs"] for g in range(4)], axis=0)
        for b in range(B)
    ]

    # ---- launch 2: out-proj + residual + LayerNorm, sharded (batch x seq/4) ----
    wo_bf = np.ascontiguousarray((Wo / WSCALE).astype(BF16))
    bo_bf = np.ascontiguousarray(bo.astype(BF16))
    in_maps2 = []
    for c in range(NCORES):
        b, q = c // 4, c % 4
        sl = slice(q * SQR, (q + 1) * SQR)
        # reciprocal of each head's rowsum, replicated across its 64 aT
        # rows: rsr[p, c, :] = 1 / rs[2c + (p >= 64), :]
        rc_q = (1.0 / rs_full[b][:, sl].astype(F32)).astype(BF16)  # [16, 512]
        rsr = np.ascontiguousarray(
            np.repeat(rc_q, HD, axis=0).reshape(KC, 128, SQR).transpose(1, 0, 2)
        )
        in_maps2.append(
            {
                "aT": np.ascontiguousarray(attT_full[b][:, sl]),
                "rsr": rsr,
                "wo": wo_bf,
                "bo": bo_bf,
                "resid": np.ascontiguousarray(query[b, sl, :].astype(BF16)),
                "gamma": gamma,
                "beta": beta,
            }
        )
    nc2 = _get("l2")
    r2 = run_bass_kernel_spmd(nc2, in_maps2, core_ids=list(range(NCORES)), trace=TRACE)
    if TRACE:
        LAST_EXEC_NS.append(r2.exec_time_ns)

    out = np.empty((B, S, D), dtype=F32)
    for c in range(NCORES):
        b, q = c // 4, c % 4
        out[b, q * SQR : (q + 1) * SQR, :] = r2.results[c]["out"]
    return out
